# revision 20
# baseline (speedup 1.0000x reference)
"""DendriticAttentionNeuron fused Bass/Tile kernel for Trainium2 (8 NeuronCores).

Strategy: data-parallel over batch (1024 rows/core, zero collectives).
Per core, everything is computed in a "transposed" activation layout
yT[out_unit, batch] so that per-head constants are per-partition scalars:

  phase T: load host-pretransposed xT [4096,1024] into SBUF (resident, bf16)
  phase Q: qT = tanh(invtau*(ff + sigmoid(g+b)*ctx))  (3 gemm chains, fp32 psum)
  phase K: same for kT; scores[h,b] accumulated via 0/1-mask matmuls of qT*kT
  phase S: softmax over the 16 heads (PE transposes to put heads on free dim)
  phase V: vT = relu(invtau*ff); combT = expand(attn) * vT
  phase O: out = combT.T @ Wout (natural [b,j] output), fused spike/v_new

Weights are host-swizzled so each (plane, o-chunk, i-half) loads with one DMA
of 4KB-contiguous per-partition lines; weights stream through SBUF exactly
once. All matmul inputs bf16 (fp32 accumulation in PSUM); softmax in fp32.
Measured: ~970us HW exec (~90% of the per-core bf16 TensorE roofline),
end-to-end precision vs the fp32 reference ~4e-3 scale-relative absmax.
"""
import numpy as np
import ml_dtypes

B, IN, H, HD = 8192, 4096, 16, 64
HID = H * HD            # 1024
N_CORES = 8
BPC = B // N_CORES      # 1024 rows per core
P = 128
NI = IN // P            # 32 i-chunks
NO = HID // P           # 8 o-chunks
NBH = 2                 # batch halves of 512 (matmul free dim)
BH = BPC // NBH         # 512
NJS = IN // 512         # 8 output j-slices
NBC = BPC // P          # 8 batch chunks of 128

TAU_MIN, TAU_MAX, TAU_SOMA = 2.0, 32.0, 2.0
V_TH = 1.0
SURROGATE_ALPHA = 4.0

_CACHE = {}


def _build_nc(score_scale: float):
    import concourse.mybir as mybir
    import concourse.tile as tile
    from concourse import bacc

    bf16 = mybir.dt.bfloat16
    f32 = mybir.dt.float32
    AF = mybir.ActivationFunctionType
    OP = mybir.AluOpType

    nc = bacc.Bacc("TRN2", target_bir_lowering=False, debug=False,
                   num_devices=N_CORES)

    # x pre-transposed on host: xT[ci, p, b] = x[b, ci*128+p]
    xT_d = nc.dram_tensor("xT", [NI, P, BPC], bf16, kind="ExternalInput").ap()
    # weights pre-swizzled on host: wcat[j, c, p, ic, o] = W_j[ic*128+p, c*128+o]
    # -> one DMA per (j, o-chunk, i-half) with 4KB contiguous lines/partition
    wcat_d = nc.dram_tensor("wcat", [7, NO, P, NI, P], bf16, kind="ExternalInput").ap()
    wout_d = nc.dram_tensor("wout", [HID, IN], bf16, kind="ExternalInput").ap()
    ident_d = nc.dram_tensor("ident", [P, P], f32, kind="ExternalInput").ap()
    maskS_d = nc.dram_tensor("maskS", [P, NO, H], bf16, kind="ExternalInput").ap()
    maskE_d = nc.dram_tensor("maskE", [H, NO, P], bf16, kind="ExternalInput").ap()
    invtau_d = nc.dram_tensor("invtau", [P, NO], f32, kind="ExternalInput").ap()
    bq_d = nc.dram_tensor("bq", [P, NO], f32, kind="ExternalInput").ap()
    bk_d = nc.dram_tensor("bk", [P, NO], f32, kind="ExternalInput").ap()

    spike_d = nc.dram_tensor("spike", [BPC, IN], f32, kind="ExternalOutput").ap()
    vnew_d = nc.dram_tensor("vnew", [BPC, IN], f32, kind="ExternalOutput").ap()

    with tile.TileContext(nc) as tc:
        with (
            tc.tile_pool(name="const", bufs=1) as cpool,
            tc.tile_pool(name="big", bufs=1) as big,
            tc.tile_pool(name="qkpool", bufs=2) as qkpool,
            tc.tile_pool(name="wts", bufs=6) as wts,
            tc.tile_pool(name="wop", bufs=2) as wop,
            tc.tile_pool(name="tmp", bufs=6) as tmp,
            tc.tile_pool(name="prodp", bufs=2) as prodp,
            tc.tile_pool(name="outp", bufs=6) as outp,
            tc.tile_pool(name="smp", bufs=4) as smp,
            tc.tile_pool(name="ps", bufs=6, space="PSUM") as ps,
            tc.tile_pool(name="pssc", bufs=1, space="PSUM") as pssc,
        ):
            # ---- constants ----
            ident = cpool.tile([P, P], f32)
            nc.sync.dma_start(out=ident[:], in_=ident_d[:])
            maskS = cpool.tile([P, NO, H], bf16)
            nc.sync.dma_start(out=maskS[:], in_=maskS_d[:])
            maskE = cpool.tile([H, NO, P], bf16)
            nc.sync.dma_start(out=maskE[:], in_=maskE_d[:])
            invtau = cpool.tile([P, NO], f32)
            nc.sync.dma_start(out=invtau[:], in_=invtau_d[:])
            bq = cpool.tile([P, NO], f32)
            nc.sync.dma_start(out=bq[:], in_=bq_d[:])
            bk = cpool.tile([P, NO], f32)
            nc.sync.dma_start(out=bk[:], in_=bk_d[:])
            spike_bias = cpool.tile([P, 1], f32)
            nc.vector.memset(spike_bias[:], float(-SURROGATE_ALPHA * V_TH))

            # ---- phase T: load pre-transposed x into SBUF ----
            xT = big.tile([P, NI, BPC], bf16)   # 64 KiB/partition
            for ci in range(NI):
                nc.scalar.dma_start(out=xT[:, ci, :], in_=xT_d[ci])

            qT = qkpool.tile([P, NO, BPC], bf16, tag="qk")
            kT = qkpool.tile([P, NO, BPC], bf16, tag="qk")
            vT = big.tile([P, NO, BPC], bf16)

            scores = pssc.tile([H, BPC], f32, tag="sc")  # 2 PSUM banks

            NIH = NI // 2   # i-chunks per half-slab

            def gemm_phase(j_list, epi):
                """j_list: weight planes; epi(c, bh, accs) consumes psums."""
                for c in range(NO):
                    accs = [[None] * NBH for _ in j_list]
                    for jj in range(len(j_list)):
                        for bh in range(NBH):
                            accs[jj][bh] = ps.tile([P, BH], f32, tag="mm",
                                                   bufs=6, name=f"acc{jj}{bh}")
                    for half in range(2):
                        wslabs = []
                        for jj, j in enumerate(j_list):
                            wt = wts.tile([P, NIH, P], bf16, tag="w", bufs=6,
                                          name="wt")
                            src = wcat_d[j, c, :,
                                         half * NIH:(half + 1) * NIH, :]
                            if c == 0 and half == 0 and j_list[0] == 0:
                                for q4 in range(4):
                                    qs = slice(q4 * (NIH // 4),
                                               (q4 + 1) * (NIH // 4))
                                    nc.sync.dma_start(out=wt[:, qs, :],
                                                      in_=src[:, qs, :])
                            else:
                                nc.sync.dma_start(out=wt[:], in_=src)
                            wslabs.append(wt)
                        for il in range(NIH):
                            i = half * NIH + il
                            for jj in range(len(j_list)):
                                for bh in range(NBH):
                                    nc.tensor.matmul(
                                        accs[jj][bh][:], wslabs[jj][:, il, :],
                                        xT[:, i, bh * BH:(bh + 1) * BH],
                                        start=(i == 0), stop=(i == NI - 1))
                    for bh in range(NBH):
                        epi(c, bh, [accs[jj][bh] for jj in range(len(j_list))])

            # ---- phase Q ----
            def epi_q(c, bh, acc):
                ff, gg, cc = acc
                sl = slice(bh * BH, (bh + 1) * BH)
                sig = tmp.tile([P, BH], f32, tag="t", bufs=6, name="sig")
                nc.scalar.activation(sig[:], gg[:], AF.Sigmoid,
                                     bias=bq[:, c:c + 1])
                t1 = tmp.tile([P, BH], f32, tag="t", bufs=6, name="t1")
                nc.vector.tensor_tensor(out=t1[:], in0=sig[:], in1=cc[:],
                                        op=OP.mult)
                t2 = tmp.tile([P, BH], f32, tag="t", bufs=6, name="t2")
                nc.vector.tensor_tensor(out=t2[:], in0=t1[:], in1=ff[:],
                                        op=OP.add)
                nc.scalar.activation(qT[:, c, sl], t2[:], AF.Tanh,
                                     scale=invtau[:, c:c + 1])

            gemm_phase([0, 1, 2], epi_q)

            # ---- phase K (+ score accumulation) ----
            def epi_k(c, bh, acc):
                ff, gg, cc = acc
                sl = slice(bh * BH, (bh + 1) * BH)
                sig = tmp.tile([P, BH], f32, tag="t", bufs=6, name="sigk")
                nc.scalar.activation(sig[:], gg[:], AF.Sigmoid,
                                     bias=bk[:, c:c + 1])
                t1 = tmp.tile([P, BH], f32, tag="t", bufs=6, name="t1k")
                nc.vector.tensor_tensor(out=t1[:], in0=sig[:], in1=cc[:],
                                        op=OP.mult)
                t2 = tmp.tile([P, BH], f32, tag="t", bufs=6, name="t2k")
                nc.vector.tensor_tensor(out=t2[:], in0=t1[:], in1=ff[:],
                                        op=OP.add)
                nc.scalar.activation(kT[:, c, sl], t2[:], AF.Tanh,
                                     scale=invtau[:, c:c + 1])
                prod = prodp.tile([P, BH], bf16, tag="p", bufs=2, name="prod")
                nc.vector.tensor_tensor(out=prod[:], in0=qT[:, c, sl],
                                        in1=kT[:, c, sl], op=OP.mult)
                nc.tensor.matmul(scores[:, sl], maskS[:, c, :], prod[:],
                                 start=(c == 0), stop=(c == NO - 1))

            gemm_phase([3, 4, 5], epi_k)

            # ---- phase S: softmax over heads ----
            scores_sb = smp.tile([H, BPC], f32, tag="ssb", bufs=1)
            nc.scalar.activation(scores_sb[:], scores[:], AF.Copy,
                                 scale=float(score_scale))
            attnT = smp.tile([H, BPC], bf16, tag="att", bufs=1)
            for bt in range(NBC):
                sl = slice(bt * P, (bt + 1) * P)
                tp = ps.tile([P, H], f32, tag="mm", bufs=6, name="tp")
                nc.tensor.transpose(tp[:], scores_sb[:, sl], ident[:H, :H])
                ex = smp.tile([P, H], f32, tag="sm", bufs=4, name="ex")
                nc.scalar.activation(ex[:], tp[:], AF.Exp)
                ssum = smp.tile([P, 1], f32, tag="sms", bufs=4, name="ssum")
                nc.vector.reduce_sum(out=ssum[:], in_=ex[:],
                                     axis=mybir.AxisListType.X)
                rec = smp.tile([P, 1], f32, tag="sms", bufs=4, name="rec")
                nc.vector.reciprocal(rec[:], ssum[:])
                at = smp.tile([P, H], f32, tag="sm", bufs=4, name="at")
                nc.vector.tensor_scalar_mul(at[:], ex[:], rec[:])
                tp2 = ps.tile([H, P], f32, tag="mm", bufs=6, name="tp2")
                nc.tensor.transpose(tp2[:], at[:], ident[:])
                nc.vector.tensor_copy(attnT[:, sl], tp2[:])

            # ---- phase V: v gemm, expand attn, combine ----
            combT = qkpool.tile([P, NO, BPC], bf16, tag="qk")

            def epi_v(c, bh, acc):
                sl = slice(bh * BH, (bh + 1) * BH)
                nc.scalar.activation(vT[:, c, sl], acc[0][:], AF.Relu,
                                     scale=invtau[:, c:c + 1])

            gemm_phase([6], epi_v)

            # expand attn to per-unit scale and combine with vT
            for c in range(NO):
                for bh in range(NBH):
                    sl = slice(bh * BH, (bh + 1) * BH)
                    exp_ps = ps.tile([P, BH], f32, tag="mm", bufs=6,
                                     name="expps")
                    nc.tensor.matmul(exp_ps[:], maskE[:, c, :], attnT[:, sl],
                                     start=True, stop=True)
                    nc.vector.tensor_tensor(out=combT[:, c, sl],
                                            in0=exp_ps[:],
                                            in1=vT[:, c, sl], op=OP.mult)

            # ---- phase O: output projection + spike/v_new ----
            # j-slices in pairs so each combT weight-load feeds 2 matmuls
            for jp in range(NJS // 2):
                wos = []
                for quarter in range(4):
                    wo = wop.tile([P, 2, 1024], bf16, tag="wo", bufs=6,
                                  name="wo")
                    for col in range(2):
                        co = quarter * 2 + col
                        nc.sync.dma_start(
                            out=wo[:, col, :],
                            in_=wout_d[co * P:(co + 1) * P,
                                       jp * 1024:(jp + 1) * 1024])
                    wos.append(wo)
                for bc in range(NBC):
                    pos = [ps.tile([P, 512], f32, tag="mm", bufs=6,
                                   name=f"po{h}") for h in range(2)]
                    for co in range(NO):
                        for h in range(2):
                            nc.tensor.matmul(
                                pos[h][:],
                                combT[:, co, bc * P:(bc + 1) * P],
                                wos[co // 2][:, co % 2,
                                             h * 512:(h + 1) * 512],
                                start=(co == 0), stop=(co == NO - 1))
                    for h in range(2):
                        js = jp * 2 + h
                        po = pos[h]
                        spk = outp.tile([P, 512], f32, tag="o", bufs=6,
                                        name="spk")
                        nc.scalar.activation(
                            spk[:], po[:], AF.Sigmoid,
                            scale=float(SURROGATE_ALPHA / TAU_SOMA),
                            bias=spike_bias[:])
                        vnw = outp.tile([P, 512], f32, tag="o", bufs=6,
                                        name="vnw")
                        nc.vector.scalar_tensor_tensor(
                            out=vnw[:], in0=po[:], scalar=float(1.0 / TAU_SOMA),
                            in1=spk[:], op0=OP.mult, op1=OP.subtract)
                        nc.sync.dma_start(
                            out=spike_d[bc * P:(bc + 1) * P,
                                        js * 512:(js + 1) * 512], in_=spk[:])
                        nc.sync.dma_start(
                            out=vnew_d[bc * P:(bc + 1) * P,
                                       js * 512:(js + 1) * 512], in_=vnw[:])

    nc.finalize()
    return nc


def _host_consts():
    bf16 = ml_dtypes.bfloat16
    taus = np.logspace(np.log10(TAU_MIN), np.log10(TAU_MAX), H).astype(np.float32)
    inv_tau = 1.0 / taus                       # [H]
    pidx = np.arange(P)
    ident = np.eye(P, dtype=np.float32)
    maskS = np.zeros((P, NO, H), dtype=np.float32)
    maskE = np.zeros((H, NO, P), dtype=np.float32)
    invtau_pk = np.zeros((P, NO), dtype=np.float32)
    for c in range(NO):
        heads = 2 * c + pidx // HD             # [P] global head index
        maskS[pidx, c, heads] = 1.0
        maskE[heads, c, pidx] = 1.0
        invtau_pk[:, c] = inv_tau[heads]
    return ident, maskS.astype(bf16), maskE.astype(bf16), invtau_pk


def _pack_bias(b):  # b: [H, HD] -> [P, NO]
    out = np.zeros((P, NO), dtype=np.float32)
    pidx = np.arange(P)
    for c in range(NO):
        heads = 2 * c + pidx // HD
        out[:, c] = np.asarray(b, np.float32)[heads, pidx % HD]
    return out


def kernel(**inputs):
    from concourse.bass_utils import run_bass_kernel_spmd

    bf16 = ml_dtypes.bfloat16
    x = np.ascontiguousarray(np.asarray(inputs["x"], dtype=np.float32))
    temperature = float(np.asarray(inputs["temperature"], dtype=np.float32))
    score_scale = 1.0 / (np.sqrt(HD) * temperature)

    key = round(score_scale, 12)
    if key not in _CACHE:
        _CACHE[key] = _build_nc(score_scale)
    nc = _CACHE[key]

    # weights: [H, IN, HD] -> [IN, H*HD], stacked in q_ff,q_gate,q_ctx,k_ff,
    # k_gate,k_ctx,v_ff order to match wcat plane indices in the kernel
    w_names = ["Wq_ff", "Wq_gate", "Wq_ctx", "Wk_ff", "Wk_gate", "Wk_ctx", "Wv_ff"]
    # [H,IN,HD] -> [IN, HID] -> swizzle to [NO, P, NI, P]:
    #   wcat[j, c, p, ic, o] = Wj[ic*128+p, c*128+o]
    wcat = np.stack([
        np.asarray(inputs[n], np.float32).transpose(1, 0, 2).reshape(IN, HID)
          .reshape(NI, P, NO, P).transpose(2, 1, 0, 3)
        for n in w_names
    ]).astype(bf16)
    wcat = np.ascontiguousarray(wcat)
    wout = np.asarray(inputs["Wout"], np.float32).astype(bf16)

    ident, maskS, maskE, invtau_pk = _host_consts()
    bq = _pack_bias(inputs["bq_gate"])
    bk = _pack_bias(inputs["bk_gate"])

    xb = x.astype(bf16).reshape(N_CORES, BPC, IN)
    # per-core transposed layout: xT[ci, p, b] = x[b, ci*128+p]
    xTb = [np.ascontiguousarray(xb[c].T.reshape(NI, P, BPC)) for c in range(N_CORES)]
    in_maps = []
    for c in range(N_CORES):
        in_maps.append({
            "xT": xTb[c], "wcat": wcat, "wout": wout, "ident": ident,
            "maskS": maskS, "maskE": maskE, "invtau": invtau_pk,
            "bq": bq, "bk": bk,
        })

    res = run_bass_kernel_spmd(nc, in_maps, list(range(N_CORES)))
    kernel.last_results = res
    spike = np.concatenate([res.results[c]["spike"] for c in range(N_CORES)], axis=0)
    vnew = np.concatenate([res.results[c]["vnew"] for c in range(N_CORES)], axis=0)
    return (spike, vnew)
